# revision 3
# baseline (speedup 1.0000x reference)
"""Trainium2 Bass kernel for nn_Decoder (mlp3 + segment_sum decoder), 8 cores.

Strategy (data-parallel over nodes N, segment-aligned shard boundaries):
  - Host: shard rows so core c owns whole segments [128c, 128(c+1)); transpose
    x to [H, rows] so stage-1 matmul lhsT comes straight from HBM; precompute
    folded biases (the ssp "-ln2" shift folded into the next stage's bias /
    the final projection's per-segment count correction).
  - Device (per core, SPMD): for each 128-row subtile, 5 passes (4 proc t's +
    enc) x 3 stages of: matmul (fp16, weights as moving operand), LayerNorm
    stats via bn_stats/bn_aggr, softplus as Exp (fused scale/bias = LN
    normalize) then Ln(x+1), transpose between stages via matmul-by-identity,
    and a one-hot segment matmul accumulating pooled sums in PSUM.
  - Final tiny projection (pooled @ wp/we + consts) on device; host gathers
    the per-core [128 segs, T] outputs into [T, G].
"""
import sys
sys.path.insert(0, "/opt/trn_rl_repo")
import numpy as np

T, N, H, G = 4, 100000, 128, 1024
NCORES = 8
SEG_PER_CORE = G // NCORES        # 128
LN2 = float(np.log(2.0))
EPS = 1e-5
P = 128

_cache = {}


def _build(nsub, use_bias0p, use_bias0e):
    import concourse.bacc as bacc
    import concourse.tile as tile
    from concourse import mybir
    F16, F32 = mybir.dt.float16, mybir.dt.float32
    AF = mybir.ActivationFunctionType
    OP = mybir.AluOpType

    nloc = nsub * P
    nc = bacc.Bacc("TRN2", target_bir_lowering=False, debug=False,
                   enable_asserts=True, num_devices=NCORES)

    xt_proc = nc.dram_tensor("xt_proc", [T, H, nloc], F32, kind="ExternalInput").ap()
    xt_enc = nc.dram_tensor("xt_enc", [H, nloc], F32, kind="ExternalInput").ap()
    batch_loc = nc.dram_tensor("batch_loc", [nsub, P], F32, kind="ExternalInput").ap()
    pw16 = nc.dram_tensor("pw16", [3, H, H], F16, kind="ExternalInput").ap()
    ew16 = nc.dram_tensor("ew16", [3, H, H], F16, kind="ExternalInput").ap()
    pb16 = nc.dram_tensor("pb16", [3, H], F16, kind="ExternalInput").ap()
    eb16 = nc.dram_tensor("eb16", [3, H], F16, kind="ExternalInput").ap()
    ident = nc.dram_tensor("ident", [H, H], F16, kind="ExternalInput").ap()
    wp4b = nc.dram_tensor("wp4b", [P, T * H], F32, kind="ExternalInput").ap()
    web = nc.dram_tensor("web", [P, H], F32, kind="ExternalInput").ap()
    kvec = nc.dram_tensor("kvec", [P, 1], F32, kind="ExternalInput").ap()
    res = nc.dram_tensor("res", [P, T], F32, kind="ExternalOutput").ap()

    with tile.TileContext(nc) as tc:
        import contextlib
        with contextlib.ExitStack() as ctx:
            singles = ctx.enter_context(tc.tile_pool(name="singles", bufs=1))
            xload = ctx.enter_context(tc.tile_pool(name="xload", bufs=3))
            work = ctx.enter_context(tc.tile_pool(name="work", bufs=3))
            stat = ctx.enter_context(tc.tile_pool(name="stat", bufs=4))
            zpool = ctx.enter_context(tc.tile_pool(name="zp", bufs=2, space="PSUM"))
            etpool = ctx.enter_context(tc.tile_pool(name="etp", bufs=2, space="PSUM"))
            mis = ctx.enter_context(tc.tile_pool(name="mis", bufs=2, space="PSUM"))
            acc = ctx.enter_context(tc.tile_pool(name="acc", bufs=1, space="PSUM"))

            # --- one-time constants ---
            w16 = []   # [family][stage] -> [H, H] fp16 tile
            for fam, src in (("p", pw16), ("e", ew16)):
                fam_tiles = []
                for s in range(3):
                    wt = singles.tile([H, H], F16, tag=f"w{fam}{s}")
                    nc.sync.dma_start(out=wt, in_=src[s])
                    fam_tiles.append(wt)
                w16.append(fam_tiles)
            b16 = []
            for fam, src in (("p", pb16), ("e", eb16)):
                fam_tiles = []
                for s in range(3):
                    bt = singles.tile([1, H], F16, tag=f"b{fam}{s}")
                    nc.sync.dma_start(out=bt, in_=src[s:s + 1, :])
                    fam_tiles.append(bt)
                b16.append(fam_tiles)
            i16 = singles.tile([H, H], F16, tag="ident")
            nc.sync.dma_start(out=i16, in_=ident)
            ones16 = singles.tile([1, H], F16, tag="ones")
            nc.vector.memset(ones16, 1.0)
            eps_t = singles.tile([P, 1], F32, tag="eps")
            nc.vector.memset(eps_t, EPS)
            iota = singles.tile([P, P], F32, tag="iota")
            nc.gpsimd.iota(iota, pattern=[[1, P]], base=0, channel_multiplier=0,
                           allow_small_or_imprecise_dtypes=True)
            wp4t = singles.tile([P, T * H], F32, tag="wp4")
            nc.sync.dma_start(out=wp4t, in_=wp4b)
            webt = singles.tile([P, H], F32, tag="web")
            nc.sync.dma_start(out=webt, in_=web)
            kvt = singles.tile([P, 1], F32, tag="kv")
            nc.sync.dma_start(out=kvt, in_=kvec)

            # --- persistent pooled accumulators in PSUM ---
            pp = acc.tile([P, T, H], F32, tag="pp")      # proc pooled, one bank
            pe = acc.tile([P, H], F32, tag="pe")         # enc pooled

            use_bias = [[use_bias0p, True, True], [use_bias0e, True, True]]

            for j in range(nsub):
                blt = xload.tile([P, 1], F32, tag="bl")
                nc.sync.dma_start(out=blt, in_=batch_loc[j:j + 1, :].rearrange("a p -> p a"))
                s16 = work.tile([P, P], F16, tag="s16")
                nc.vector.tensor_scalar(out=s16, in0=iota, scalar1=blt, scalar2=0.0,
                                        op0=OP.is_equal, op1=OP.bypass)

                # load the 5 stage-1 lhsT tiles ([h, rows] fp16, cast in DMA)
                lhs = []
                for p in range(5):
                    xt = xload.tile([H, P], F16, tag=f"x{p}")
                    if p < T:
                        nc.gpsimd.dma_start(out=xt, in_=xt_proc[p, :, j * P:(j + 1) * P])
                    else:
                        nc.gpsimd.dma_start(out=xt, in_=xt_enc[:, j * P:(j + 1) * P])
                    lhs.append(xt)

                for s in range(3):
                    z4 = zpool.tile([P, T, H], F32, tag="z4")
                    zm = mis.tile([P, 2, H], F32, tag="zm")
                    ze = zm[:, 0, :]
                    zs = [z4[:, t, :] for t in range(T)] + [ze]
                    for p in range(5):
                        fam = 0 if p < T else 1
                        has_b = use_bias[fam][s]
                        nc.tensor.matmul(zs[p], lhsT=lhs[p], rhs=w16[fam][s],
                                         start=True, stop=not has_b)
                        if has_b:
                            nc.tensor.matmul(zs[p], lhsT=ones16, rhs=b16[fam][s],
                                             start=False, stop=True)
                    # stats
                    mv5 = stat.tile([P, 5, 2], F32, tag="mv5")
                    for p in range(5):
                        st = stat.tile([P, 6], F32, tag=f"st{p}")
                        nc.vector.bn_stats(out=st, in_=zs[p])
                        nc.vector.bn_aggr(out=mv5[:, p, :], in_=st)
                    sd5 = stat.tile([P, 5], F32, tag="sd5")
                    nc.scalar.activation(out=sd5, in_=mv5[:, :, 1], func=AF.Sqrt,
                                         bias=eps_t, scale=1.0)
                    r5 = stat.tile([P, 5], F32, tag="r5")
                    nc.vector.reciprocal(out=r5, in_=sd5)
                    nmr5 = stat.tile([P, 5], F32, tag="nmr5")
                    nc.vector.tensor_tensor(out=nmr5, in0=mv5[:, :, 0], in1=r5,
                                            op=OP.mult)
                    nc.vector.tensor_scalar(out=nmr5, in0=nmr5, scalar1=-1.0,
                                            scalar2=0.0, op0=OP.mult, op1=OP.bypass)
                    # exp (fused LN normalize), fp16 out
                    e5 = work.tile([P, 5, H], F16, tag="e5")
                    for p in range(5):
                        nc.scalar.activation(out=e5[:, p, :], in_=zs[p], func=AF.Exp,
                                             bias=nmr5[:, p:p + 1], scale=r5[:, p:p + 1])
                    if s < 2:
                        # transpose E, then a^T = Ln(E^T + 1) feeds next stage
                        et4 = etpool.tile([P, T, H], F32, tag="et4")
                        ete = zm[:, 1, :]
                        ets = [et4[:, t, :] for t in range(T)] + [ete]
                        for p in range(5):
                            nc.tensor.matmul(ets[p], lhsT=e5[:, p, :], rhs=i16,
                                             start=True, stop=True)
                        at5 = work.tile([P, 5, H], F16, tag="at5")
                        nc.scalar.activation(out=at5[:, 0:T, :], in_=et4, func=AF.Ln,
                                             bias=1.0, scale=1.0)
                        nc.scalar.activation(out=at5[:, T, :], in_=ete, func=AF.Ln,
                                             bias=1.0, scale=1.0)
                        lhs = [at5[:, p, :] for p in range(5)]
                    else:
                        # rows-in-partition softplus output feeds segment matmul
                        a5 = work.tile([P, 5, H], F16, tag="a5")
                        nc.scalar.activation(out=a5, in_=e5, func=AF.Ln,
                                             bias=1.0, scale=1.0)
                        # one accumulation group per PSUM bank: open on the
                        # first matmul touching the bank, close on the last.
                        for p in range(5):
                            out_ap = pp[:, p, :] if p < T else pe
                            first = (j == 0) and (p == 0 or p == T)
                            last = (j == nsub - 1) and (p == T - 1 or p == T)
                            nc.tensor.matmul(out_ap, lhsT=s16, rhs=a5[:, p, :],
                                             start=first, stop=last)

            # --- final projection ---
            ppf = singles.tile([P, T, H], F32, tag="ppf")
            nc.vector.tensor_copy(out=ppf, in_=pp)
            pef = singles.tile([P, H], F32, tag="pef")
            nc.vector.tensor_copy(out=pef, in_=pe)
            ppm = singles.tile([P, T * H], F32, tag="ppm")
            nc.vector.tensor_tensor(out=ppm, in0=ppf.rearrange("p a b -> p (a b)"),
                                    in1=wp4t, op=OP.mult)
            pem = singles.tile([P, H], F32, tag="pem")
            nc.vector.tensor_tensor(out=pem, in0=pef, in1=webt, op=OP.mult)
            projp = singles.tile([P, T], F32, tag="projp")
            nc.vector.reduce_sum(out=projp, in_=ppm.rearrange("p (a b) -> p a b", a=T),
                                 axis=mybir.AxisListType.X)
            proje = singles.tile([P, 1], F32, tag="proje")
            nc.vector.reduce_sum(out=proje, in_=pem, axis=mybir.AxisListType.X)
            rest = singles.tile([P, T], F32, tag="rest")
            nc.vector.tensor_scalar(out=rest, in0=projp, scalar1=proje, scalar2=kvt,
                                    op0=OP.add, op1=OP.add)
            nc.sync.dma_start(out=res, in_=rest)

    nc.compile()
    return nc


class _Runner:
    """Holds the jitted PJRT callable for repeated execution."""

    def __init__(self, nc, n_cores):
        import jax
        from jax.sharding import Mesh, PartitionSpec
        from jax.experimental.shard_map import shard_map
        from concourse import mybir
        from concourse.bass2jax import (_bass_exec_p, install_neuronx_cc_hook,
                                        partition_id_tensor)
        install_neuronx_cc_hook()
        self.jax = jax
        self.n_cores = n_cores
        partition_name = nc.partition_id_tensor.name if nc.partition_id_tensor else None
        dbg_name = nc.dbg_addr.name if nc.dbg_addr else None
        in_names, out_names, out_avals, zero_outs = [], [], [], []
        for alloc in nc.m.functions[0].allocations:
            if not isinstance(alloc, mybir.MemoryLocationSet):
                continue
            name = alloc.memorylocations[0].name
            if alloc.kind == "ExternalInput":
                if name not in (partition_name, dbg_name):
                    in_names.append(name)
            elif alloc.kind == "ExternalOutput":
                shape = tuple(alloc.tensor_shape)
                dtype = mybir.dt.np(alloc.dtype)
                out_names.append(name)
                out_avals.append(jax.core.ShapedArray(shape, dtype))
                zero_outs.append(np.zeros(shape, dtype))
        self.in_names, self.out_names = in_names, out_names
        self.out_avals, self.zero_outs = out_avals, zero_outs
        all_in = list(in_names) + list(out_names)
        if dbg_name is not None:
            all_in.append(dbg_name)
        if partition_name is not None:
            all_in.append(partition_name)

        def _body(*args):
            operands = list(args)
            if dbg_name is not None:
                operands.append(jax.numpy.zeros((1, 2), jax.numpy.uint32))
            if partition_name is not None:
                operands.append(partition_id_tensor())
            return tuple(_bass_exec_p.bind(
                *operands, out_avals=tuple(out_avals), in_names=tuple(all_in),
                out_names=tuple(out_names), lowering_input_output_aliases=(),
                sim_require_finite=True, sim_require_nnan=True, nc=nc))

        devices = jax.devices()[:n_cores]
        self.mesh = Mesh(np.asarray(devices), ("core",))
        n_io = len(in_names) + len(out_names)
        self.fn = jax.jit(
            shard_map(_body, mesh=self.mesh,
                      in_specs=(PartitionSpec("core"),) * n_io,
                      out_specs=(PartitionSpec("core"),) * len(out_names),
                      check_rep=False),
            keep_unused=True)

    def prepare(self, in_maps):
        import jax
        from jax.sharding import PartitionSpec
        n = self.n_cores
        sharding = jax.sharding.NamedSharding(self.mesh, PartitionSpec("core"))
        dev_in = [jax.device_put(
            np.concatenate([np.asarray(in_maps[c][name]) for c in range(n)], axis=0),
            sharding) for name in self.in_names]
        dev_zero = [jax.device_put(
            np.zeros((n * z.shape[0], *z.shape[1:]), z.dtype), sharding)
            for z in self.zero_outs]
        return dev_in, dev_zero

    def run(self, handle):
        dev_in, dev_zero = handle
        outs = self.fn(*dev_in, *dev_zero)
        self.jax.block_until_ready(outs)
        return outs

    def results(self, outs):
        n = self.n_cores
        return [{name: np.asarray(outs[i]).reshape(n, *self.out_avals[i].shape)[c]
                 for i, name in enumerate(self.out_names)} for c in range(n)]


def _prep_inputs(x_proc, x_enc, batch, pW, pb, pg, pbt, eW, eb, eg, ebt,
                 wp, bp, we, be):
    """Host-side sharding + precomputation. Returns (in_maps, meta)."""
    x_proc = np.asarray(x_proc, dtype=np.float32)
    x_enc = np.asarray(x_enc, dtype=np.float32)
    batch = np.asarray(batch).astype(np.int64)
    pW = np.asarray(pW, dtype=np.float32)
    eW = np.asarray(eW, dtype=np.float32)
    pb = np.asarray(pb, dtype=np.float32)
    eb = np.asarray(eb, dtype=np.float32)
    wp = np.asarray(wp, dtype=np.float32).reshape(H)
    we = np.asarray(we, dtype=np.float32).reshape(H)
    bp = float(np.asarray(bp).reshape(-1)[0])
    be = float(np.asarray(be).reshape(-1)[0])

    # note: reference applies gains/shifts pg,pbt,eg,ebt as LN affine params;
    # for this problem they are ones/zeros — verify and bail loudly otherwise.
    assert np.allclose(np.asarray(pg), 1) and np.allclose(np.asarray(eg), 1), \
        "kernel assumes LN gain == 1"
    assert np.allclose(np.asarray(pbt), 0) and np.allclose(np.asarray(ebt), 0), \
        "kernel assumes LN shift == 0"

    splits = np.searchsorted(batch, np.arange(NCORES + 1) * SEG_PER_CORE)
    rows = splits[1:] - splits[:-1]
    nloc_raw = int(rows.max())
    nsub = max(1, (nloc_raw + P - 1) // P)
    nloc = nsub * P

    pw16 = pW.astype(np.float16)
    ew16 = eW.astype(np.float16)
    # folded biases: stage 0 bias = b; stages 1,2 get  b - ln2 * colsum(W16)
    pb_eff = np.stack([pb[0],
                       pb[1] - LN2 * pw16[1].astype(np.float32).sum(0),
                       pb[2] - LN2 * pw16[2].astype(np.float32).sum(0)])
    eb_eff = np.stack([eb[0],
                       eb[1] - LN2 * ew16[1].astype(np.float32).sum(0),
                       eb[2] - LN2 * ew16[2].astype(np.float32).sum(0)])
    use_bias0p = bool(np.abs(pb[0]).max() > 0)
    use_bias0e = bool(np.abs(eb[0]).max() > 0)

    ident = np.eye(H, dtype=np.float16)
    wp4b = np.tile(wp[None, :], (P, T)).astype(np.float32)        # [P, T*H]
    web = np.tile(we[None, :], (P, 1)).astype(np.float32)         # [P, H]

    in_maps = []
    for c in range(NCORES):
        lo, hi = int(splits[c]), int(splits[c + 1])
        n_c = hi - lo
        xtp = np.zeros((T, H, nloc), np.float32)
        xtp[:, :, :n_c] = x_proc[:, lo:hi, :].transpose(0, 2, 1)
        xte = np.zeros((H, nloc), np.float32)
        xte[:, :n_c] = x_enc[lo:hi, :].T
        bl = np.full(nloc, -1.0, np.float32)
        bl[:n_c] = (batch[lo:hi] - c * SEG_PER_CORE).astype(np.float32)
        cnt = np.zeros(SEG_PER_CORE, np.float64)
        segs, counts = np.unique(batch[lo:hi], return_counts=True)
        cnt[(segs - c * SEG_PER_CORE).astype(int)] = counts
        kv = (bp + be - LN2 * cnt * (wp.sum() + we.sum())).astype(np.float32)
        in_maps.append({
            "xt_proc": xtp, "xt_enc": xte,
            "batch_loc": bl.reshape(nsub, P),
            "pw16": pw16, "ew16": ew16,
            "pb16": pb_eff.astype(np.float16), "eb16": eb_eff.astype(np.float16),
            "ident": ident, "wp4b": wp4b, "web": web,
            "kvec": kv.reshape(P, 1),
        })
    meta = (nsub, use_bias0p, use_bias0e)
    return in_maps, meta


def get_runner(meta):
    key = meta
    if key not in _cache:
        nc = _build(*meta)
        _cache[key] = _Runner(nc, NCORES)
    return _cache[key]


def kernel(**inputs) -> np.ndarray:
    in_maps, meta = _prep_inputs(**inputs)
    runner = get_runner(meta)
    handle = runner.prepare(in_maps)
    outs = runner.run(handle)
    per_core = runner.results(outs)
    out = np.zeros((T, G), np.float32)
    for c in range(NCORES):
        out[:, c * SEG_PER_CORE:(c + 1) * SEG_PER_CORE] = per_core[c]["res"].T
    return out
